# revision 11
# baseline (speedup 1.0000x reference)
"""Trainium2 Bass kernel for a single DeBERTa-style attention head (v4).

Problem shapes (hardcoded):
  B=8, S=2048, E=768(n_embed), H=64(head)
  q = I @ Wq + bq ; k = x @ Wk + bk ; v = x @ Wv + bv
  w = (q @ k^T) / sqrt(E) ; w = where(mask==0, -1e9, w)
  scores = softmax(w, axis=-1) ; out = scores @ v

Sharding: data-parallel over batch B across the 8 NeuronCores (one batch
element per core, identical SPMD program).

The kernel is paced by the exp chain on ACT (32 x [128,1024] windows
~= 35us); the DMA stream (~42us of engine-time for 10.8MB reads +
cast-amplified mask writes) overlaps it almost fully because window
order follows data arrival:
  * All inputs are host-packed bf16 partition-major so every DMA moves
    6-16KB contiguous runs per partition (small descriptors capped v2 at
    ~250GB/s).
  * Sync-queue order: W, ITa, XTb0, ITb, XTb1..3.  Windows run r-major in
    "super-pairs" (chunk pairs 0-1 q-half0, pairs 0-1 q-half1, pairs 2-3
    ...), so pair r only needs XT/mask through chunk 2r+1 - prerequisites
    arrive progressively instead of all-up-front.
  * Mask chunk-group cast-DMAs (SWDGE, uint8->bf16) are gated behind
    input blocks by tiny gpsimd reduces (WAW dep on the mask tile), so
    they cannot starve the input stream early and land just-in-time.
  * bk dropped (softmax shift-invariant), bv applied on host, bq folded
    into the q PSUM->SBUF copy.  Device returns unnormalized context^T
    + denominator row ([65, S] fp32); host divides.
  * Scores (K=64) run as row-tiled pairs (even chunks' kT on partitions
    0-63, odd on 64-127; qT duplicated onto both halves via small
    SBUF->SBUF DMAs on the scalar HWDGE ring) - two chunks per PE pass.
  * ctx is vA-stationary (16 LDWEIGHTS, not 256); vA carries a ones
    column so accumulator row 64 is the softmax denominator.
  * PSUM: score pool 2x[128,1024] (4 banks) + projection pool (4 banks,
    reused by the ctx accumulator once projections drain) = 8 banks.
    The q half-b projection borrows score buffers mid-pipeline.
"""

import math
from contextlib import ExitStack

import numpy as np

import concourse.bass as bass
import concourse.tile as tile
import concourse.mybir as mybir
from concourse import bacc
from concourse.bass_utils import run_bass_kernel_spmd

B, S, E, H = 8, 2048, 768, 64
N_CORES = 8
SC = S // 128   # 16 seq chunks
EC = E // 128   # 6 embed chunks
SCALE = 1.0 / math.sqrt(E)

F32 = mybir.dt.float32
BF16 = mybir.dt.bfloat16
AF = mybir.ActivationFunctionType
ALU = mybir.AluOpType
AX = mybir.AxisListType

_cache = {}


def _build_program():
    nc = bacc.Bacc("TRN2", target_bir_lowering=False, debug=False)

    # I^T in two seq-halves, x^T in four seq-blocks, partition-major
    dIT = nc.dram_tensor("IT", [2, 128, EC, 1024], BF16, kind="ExternalInput")
    dXT = nc.dram_tensor("XT", [4, 128, EC, 512], BF16, kind="ExternalInput")
    # [p, chunk, q]: a 4-chunk group is an 8KB contiguous run per partition
    dmT = nc.dram_tensor("maskT", [128, SC, S], mybir.dt.uint8, kind="ExternalInput")
    dW = nc.dram_tensor("Wpack", [E, 3 * H], BF16, kind="ExternalInput")
    dbq = nc.dram_tensor("bq2", [128, 1], F32, kind="ExternalInput")
    # rows 0-63: unnormalized context^T; row 64: softmax denominator
    dout = nc.dram_tensor("out", [H + 1, S], F32, kind="ExternalOutput")

    with tile.TileContext(nc) as tc, ExitStack() as ctx:
        singles = ctx.enter_context(tc.tile_pool(name="singles", bufs=1))

        IT = singles.tile([128, 2, EC, 1024], BF16, tag="IT")
        XT = singles.tile([128, 4, EC, 512], BF16, tag="XT")
        w_all = singles.tile([128, EC, 3 * H], BF16, tag="Wpack")
        bq2 = singles.tile([128, 1], F32, tag="bq2")
        qT2 = singles.tile([128, S], BF16, tag="qT2")
        kT = singles.tile([64, S], BF16, tag="kT")
        kodd = singles.tile([128, S // 2], BF16, tag="kodd")
        vA = singles.tile([128, SC, 66], BF16, tag="vA")
        out_sb = singles.tile([H + 1, S], F32, tag="out_sb")

        # Trigger the ACT exp table load (~2.7us) while the DMAs stream.
        warm_i = singles.tile([1, 16], F32, tag="warm_i")
        warm_o = singles.tile([1, 16], BF16, tag="warm_o")
        nc.vector.memset(warm_i, 0.0)
        nc.scalar.activation(warm_o, warm_i, AF.Exp)

        nc.vector.memset(vA[:, :, 64:65], 1.0)

        # ---- input DMAs (sync HWDGE queue drains in this order) ----
        nc.sync.dma_start(
            out=w_all, in_=dW.ap().rearrange("(ec p) h -> p ec h", p=128)
        )
        nc.sync.dma_start(out=bq2, in_=dbq.ap())
        nc.sync.dma_start(out=IT[:, 0], in_=dIT.ap()[0])
        nc.sync.dma_start(out=XT[:, 0], in_=dXT.ap()[0])
        nc.sync.dma_start(out=IT[:, 1], in_=dIT.ap()[1])
        for blk in range(1, 4):
            nc.sync.dma_start(out=XT[:, blk], in_=dXT.ap()[blk])

        # ---- mask cast-DMAs in 4-chunk groups, arrival-gated ----
        mpool = ctx.enter_context(tc.tile_pool(name="mpool", bufs=3))
        gates = [IT[0:1, 0, 0, 0:1], XT[0:1, 1, 0, 0:1], XT[0:1, 2, 0, 0:1],
                 XT[0:1, 3, 0, 0:1]]
        m_groups = []
        for g in range(4):
            mt = mpool.tile([128, 4, S], BF16, tag="m")
            nc.gpsimd.tensor_reduce(mt[0:1, 0:1, 0:1], gates[g], AX.C, ALU.max)
            nc.gpsimd.dma_start(out=mt, in_=dmT.ap()[:, g * 4:(g + 1) * 4, :])
            m_groups.append(mt)

        def m_tile(c):
            return m_groups[c // 4][:, c % 4, :]

        # ---- PSUM pools ----
        wpool = ctx.enter_context(tc.tile_pool(name="wpool", bufs=2, space="PSUM"))
        proj_scope = ExitStack()
        psproj = proj_scope.enter_context(
            tc.tile_pool(name="psproj", bufs=2, space="PSUM")
        )

        def emit_q(half, mk_psq):
            for blk in range(2):
                psq = mk_psq()
                for ei in range(EC):
                    nc.tensor.matmul(
                        psq,
                        lhsT=w_all[:, ei, 0:H],
                        rhs=IT[:, half, ei, blk * 512:(blk + 1) * 512],
                        start=(ei == 0),
                        stop=(ei == EC - 1),
                    )
                sl = slice(half * 1024 + blk * 512, half * 1024 + (blk + 1) * 512)
                nc.vector.tensor_scalar(qT2[0:64, sl], psq, bq2[0:64], None, ALU.add)
            sl = slice(half * 1024, (half + 1) * 1024)
            nc.scalar.dma_start(out=qT2[64:128, sl], in_=qT2[0:64, sl])

        def emit_k(blk):
            # chunks 4*blk .. 4*blk+3
            sl = slice(blk * 512, (blk + 1) * 512)
            psk = psproj.tile([64, 512], F32, tag="pk", name="psk")
            for ei in range(EC):
                nc.tensor.matmul(
                    psk,
                    lhsT=w_all[:, ei, H:2 * H],
                    rhs=XT[:, blk, ei, :],
                    start=(ei == 0),
                    stop=(ei == EC - 1),
                )
            nc.vector.tensor_copy(kT[:, sl], psk)
            # odd chunks (local 1,3) -> partitions 64-127 of kodd at pair
            # columns [2*blk*128, (2*blk+2)*128)
            nc.scalar.dma_start(
                out=kodd[64:128, blk * 256:(blk + 1) * 256].rearrange(
                    "p (two c) -> p two c", c=128
                ),
                in_=kT[:, sl].rearrange("p (four c) -> p four c", c=128)[:, 1::2, :],
            )

        def emit_v(c):
            psv = psproj.tile([128, H], F32, tag="pv", name="psv")
            for ei in range(EC):
                nc.tensor.matmul(
                    psv,
                    lhsT=XT[:, c // 4, ei, (c % 4) * 128:(c % 4 + 1) * 128],
                    rhs=w_all[:, ei, 2 * H:3 * H],
                    start=(ei == 0),
                    stop=(ei == EC - 1),
                )
            nc.vector.tensor_copy(vA[:, c, 0:H], psv)

        # ---- score / softmax / ctx pipeline ----
        epool = ctx.enter_context(tc.tile_pool(name="epool", bufs=3))
        spool = ctx.enter_context(tc.tile_pool(name="spool", bufs=12))

        psctx_holder = {}

        def emit_window(h, r, j, c):
            wt = wpool.tile([128, 1024], F32, tag="w", name="wt")
            for qb in range(2):
                col = h * 1024 + qb * 512
                if j == 0:
                    lhsT = kT[:, c * 128:(c + 1) * 128]
                    rhs = qT2[0:64, col:col + 512]
                else:
                    lhsT = kodd[64:128, r * 128:(r + 1) * 128]
                    rhs = qT2[64:128, col:col + 512]
                nc.tensor.matmul(
                    wt[:, qb * 512:(qb + 1) * 512],
                    lhsT=lhsT, rhs=rhs, start=True, stop=True,
                )
            et = epool.tile([128, 1024], BF16, tag="e", name="et")
            nc.scalar.activation(et, wt, AF.Exp, scale=SCALE)
            st = spool.tile([128, 1024], BF16, tag="s", name="st")
            nc.vector.tensor_tensor(
                st, et, m_tile(c)[:, h * 1024:(h + 1) * 1024], ALU.mult
            )
            ctxall = psctx_holder["t"]
            for qb in range(2):
                col = h * 1024 + qb * 512
                nc.tensor.matmul(
                    ctxall[0:H + 1, col:col + 512],
                    lhsT=vA[:, c, 0:H + 1],
                    rhs=st[:, qb * 512:(qb + 1) * 512],
                    start=(r == 0 and j == 0),
                    stop=(r == SC // 2 - 1 and j == 1),
                )

        # Emission order = engine-queue order; each emit's deps follow the
        # DMA arrival order (ITa, XTb0, ITb, XTb1, XTb2, XTb3).
        emit_q(0, lambda: psproj.tile([64, 512], F32, tag="pk", name="psq"))
        emit_k(0)
        for c in range(0, 4):
            emit_v(c)
        emit_k(1)
        for c in range(4, 8):
            emit_v(c)
        emit_k(2)
        for c in range(8, 12):
            emit_v(c)
        emit_k(3)
        for c in range(12, 16):
            emit_v(c)
        proj_scope.close()

        psctx = ctx.enter_context(tc.tile_pool(name="psctx", bufs=1, space="PSUM"))
        psctx_holder["t"] = psctx.tile([128, S], F32, tag="ctxall", name="ctxall")

        for R in range(0, SC // 2, 2):
            for h in range(2):
                if R == 0 and h == 1:
                    # q half-b projection borrows score PSUM buffers; its
                    # MMs sit after the first four windows in the PE
                    # queue, by which time ITb has landed.
                    emit_q(1, lambda: wpool.tile([128, 1024], F32, tag="w",
                                                 name="psq_b")[0:64, 0:512])
                for r in (R, R + 1):
                    for j in range(2):
                        emit_window(h, r, j, 2 * r + j)

        for h in range(2):
            hs = slice(h * 1024, (h + 1) * 1024)
            nc.vector.tensor_copy(out_sb[:, hs], psctx_holder["t"][0:H + 1, hs])
            nc.sync.dma_start(out=dout.ap()[:, hs], in_=out_sb[:, hs])

    nc.compile()
    return nc


def get_program():
    if "nc" not in _cache:
        _cache["nc"] = _build_program()
    return _cache["nc"]


def make_in_maps(I, x, mask, Wq, bq, Wk, bk, Wv, bv):
    import ml_dtypes

    BF = ml_dtypes.bfloat16
    I = np.asarray(I, dtype=np.float32)
    x = np.asarray(x, dtype=np.float32)
    mask = np.asarray(mask, dtype=np.int32)
    Wpack = np.concatenate(
        [
            np.asarray(Wq, dtype=np.float32),
            np.asarray(Wk, dtype=np.float32),
            np.asarray(Wv, dtype=np.float32),
        ],
        axis=1,
    ).astype(BF)
    bq2 = np.tile(np.asarray(bq, np.float32).reshape(H, 1), (2, 1))

    def pack_blocks(a, nblk):
        # [S, E] -> [nblk, p, ec, S/nblk] partition-major seq-blocks of a^T
        w = S // nblk
        t = np.ascontiguousarray(a.T).astype(BF).reshape(EC, 128, nblk, w)
        return np.ascontiguousarray(t.transpose(2, 1, 0, 3))

    def pack_mask(m):
        # [S(q), S(k)] -> [p, chunk, q] of mask^T
        return np.ascontiguousarray(
            m.T.astype(np.uint8).reshape(SC, 128, S).transpose(1, 0, 2)
        )

    return [
        {
            "IT": pack_blocks(I[b], 2),
            "XT": pack_blocks(x[b], 4),
            "maskT": pack_mask(mask[b]),
            "Wpack": Wpack,
            "bq2": bq2,
        }
        for b in range(B)
    ]


def postprocess(raw, bv):
    """raw: [65, S] f32 (64 ctx rows + denominator). Returns [S, H] f32."""
    return (raw[0:H] / raw[H:H + 1]).T + np.asarray(bv, np.float32)


def kernel(I, x, mask, Wq, bq, Wk, bk, Wv, bv):
    nc = get_program()
    in_maps = make_in_maps(I, x, mask, Wq, bq, Wk, bk, Wv, bv)
    res = run_bass_kernel_spmd(nc, in_maps, list(range(N_CORES)))
    out = np.stack(
        [postprocess(res.results[b]["out"], bv) for b in range(B)], axis=0
    )
    return out.astype(np.float32)


# revision 13
# speedup vs baseline: 1.0865x; 1.0865x over previous
"""Trainium2 Bass kernel for a single DeBERTa-style attention head (v4).

Problem shapes (hardcoded):
  B=8, S=2048, E=768(n_embed), H=64(head)
  q = I @ Wq + bq ; k = x @ Wk + bk ; v = x @ Wv + bv
  w = (q @ k^T) / sqrt(E) ; w = where(mask==0, -1e9, w)
  scores = softmax(w, axis=-1) ; out = scores @ v

Sharding: data-parallel over batch B across the 8 NeuronCores (one batch
element per core, identical SPMD program).

The kernel is paced by the exp chain on ACT (32 x [128,1024] windows
~= 35us); the DMA stream (~42us of engine-time for 10.8MB reads +
cast-amplified mask writes) overlaps it almost fully because window
order follows data arrival:
  * All inputs are host-packed bf16 partition-major so every DMA moves
    6-16KB contiguous runs per partition (small descriptors capped v2 at
    ~250GB/s).
  * Sync-queue order: W, ITa, XTb0, ITb, XTb1..3.  Windows run r-major in
    "super-pairs" (chunk pairs 0-1 q-half0, pairs 0-1 q-half1, pairs 2-3
    ...), so pair r only needs XT/mask through chunk 2r+1 - prerequisites
    arrive progressively instead of all-up-front.
  * Mask chunk-group cast-DMAs (SWDGE, uint8->bf16) are gated behind
    input blocks by tiny gpsimd reduces (WAW dep on the mask tile), so
    they cannot starve the input stream early and land just-in-time.
  * bk dropped (softmax shift-invariant), bv applied on host, bq folded
    into the q PSUM->SBUF copy.  Device returns unnormalized context^T
    + denominator row ([65, S] fp32); host divides.
  * Scores (K=64) run as row-tiled pairs (even chunks' kT on partitions
    0-63, odd on 64-127; qT duplicated onto both halves via small
    SBUF->SBUF DMAs on the scalar HWDGE ring) - two chunks per PE pass.
  * ctx is vA-stationary (16 LDWEIGHTS, not 256); vA carries a ones
    column so accumulator row 64 is the softmax denominator.
  * PSUM: score pool 2x[128,1024] (4 banks) + projection pool (4 banks,
    reused by the ctx accumulator once projections drain) = 8 banks.
    The q half-b projection borrows score buffers mid-pipeline.
"""

import math
from contextlib import ExitStack

import numpy as np

import concourse.bass as bass
import concourse.tile as tile
import concourse.mybir as mybir
from concourse import bacc
from concourse.bass_utils import run_bass_kernel_spmd

B, S, E, H = 8, 2048, 768, 64
N_CORES = 8
SC = S // 128   # 16 seq chunks
EC = E // 128   # 6 embed chunks
SCALE = 1.0 / math.sqrt(E)

F32 = mybir.dt.float32
BF16 = mybir.dt.bfloat16
AF = mybir.ActivationFunctionType
ALU = mybir.AluOpType
AX = mybir.AxisListType

_cache = {}


def _build_program():
    nc = bacc.Bacc("TRN2", target_bir_lowering=False, debug=False)

    # I^T in two seq-halves, x^T in four seq-blocks, partition-major
    dIT = nc.dram_tensor("IT", [2, 128, EC, 1024], BF16, kind="ExternalInput")
    dXT = nc.dram_tensor("XT", [4, 128, EC, 512], BF16, kind="ExternalInput")
    # [p, chunk, q]: a 4-chunk group is an 8KB contiguous run per partition
    dmT = nc.dram_tensor("maskT", [128, SC, S], mybir.dt.uint8, kind="ExternalInput")
    dW = nc.dram_tensor("Wpack", [E, 3 * H], BF16, kind="ExternalInput")
    dbq = nc.dram_tensor("bq2", [128, 1], F32, kind="ExternalInput")
    # rows 0-63: unnormalized context^T; row 64: softmax denominator
    dout = nc.dram_tensor("out", [H + 1, S], F32, kind="ExternalOutput")

    with tile.TileContext(nc) as tc, ExitStack() as ctx:
        singles = ctx.enter_context(tc.tile_pool(name="singles", bufs=1))

        IT = singles.tile([128, 2, EC, 1024], BF16, tag="IT")
        XT = singles.tile([128, 4, EC, 512], BF16, tag="XT")
        w_all = singles.tile([128, EC, 3 * H], BF16, tag="Wpack")
        bq2 = singles.tile([128, 1], F32, tag="bq2")
        qT2 = singles.tile([128, S], BF16, tag="qT2")
        kT = singles.tile([64, S], BF16, tag="kT")
        kodd = singles.tile([128, S // 2], BF16, tag="kodd")
        vA = singles.tile([128, SC, 66], BF16, tag="vA")
        out_sb = singles.tile([H + 1, S], F32, tag="out_sb")

        # Trigger the ACT exp table load (~2.7us) while the DMAs stream.
        warm_i = singles.tile([1, 16], F32, tag="warm_i")
        warm_o = singles.tile([1, 16], BF16, tag="warm_o")
        nc.vector.memset(warm_i, 0.0)
        nc.scalar.activation(warm_o, warm_i, AF.Exp)

        nc.vector.memset(vA[:, :, 64:65], 1.0)

        # ---- input DMAs: ONE sync HWDGE FIFO carries everything in
        # consumption order, mask groups (plain uint8, no cast - the v3/v4
        # cast-DMAs doubled the mask's fabric bytes) interleaved with the
        # x^T blocks they pair with.  XTb3 is pulled ahead of mg2/mg3 so
        # the projections (and the ctx accumulator's PSUM reuse) unblock
        # before the final mask groups land.
        mpool = ctx.enter_context(tc.tile_pool(name="mpool", bufs=4))
        m_groups = [
            mpool.tile([128, 4, S], mybir.dt.uint8, tag="m", name=f"mg{g}")
            for g in range(4)
        ]
        nc.sync.dma_start(
            out=w_all, in_=dW.ap().rearrange("(ec p) h -> p ec h", p=128)
        )
        nc.sync.dma_start(out=bq2, in_=dbq.ap())
        nc.sync.dma_start(out=IT[:, 0], in_=dIT.ap()[0])
        nc.sync.dma_start(out=XT[:, 0], in_=dXT.ap()[0])
        nc.sync.dma_start(out=m_groups[0], in_=dmT.ap()[:, 0:4, :])
        nc.sync.dma_start(out=IT[:, 1], in_=dIT.ap()[1])
        nc.sync.dma_start(out=XT[:, 1], in_=dXT.ap()[1])
        nc.sync.dma_start(out=m_groups[1], in_=dmT.ap()[:, 4:8, :])
        nc.sync.dma_start(out=XT[:, 2], in_=dXT.ap()[2])
        nc.sync.dma_start(out=XT[:, 3], in_=dXT.ap()[3])
        nc.sync.dma_start(out=m_groups[2], in_=dmT.ap()[:, 8:12, :])
        nc.sync.dma_start(out=m_groups[3], in_=dmT.ap()[:, 12:16, :])

        def m_tile(c):
            return m_groups[c // 4][:, c % 4, :]

        # ---- PSUM pools ----
        wpool = ctx.enter_context(tc.tile_pool(name="wpool", bufs=2, space="PSUM"))
        proj_scope = ExitStack()
        psproj = proj_scope.enter_context(
            tc.tile_pool(name="psproj", bufs=2, space="PSUM")
        )

        def emit_q(half, mk_psq):
            for blk in range(2):
                psq = mk_psq()
                for ei in range(EC):
                    nc.tensor.matmul(
                        psq,
                        lhsT=w_all[:, ei, 0:H],
                        rhs=IT[:, half, ei, blk * 512:(blk + 1) * 512],
                        start=(ei == 0),
                        stop=(ei == EC - 1),
                    )
                sl = slice(half * 1024 + blk * 512, half * 1024 + (blk + 1) * 512)
                nc.vector.tensor_scalar(qT2[0:64, sl], psq, bq2[0:64], None, ALU.add)
            sl = slice(half * 1024, (half + 1) * 1024)
            nc.scalar.dma_start(out=qT2[64:128, sl], in_=qT2[0:64, sl])

        def emit_k(blk):
            # chunks 4*blk .. 4*blk+3
            sl = slice(blk * 512, (blk + 1) * 512)
            psk = psproj.tile([64, 512], F32, tag="pk", name="psk")
            for ei in range(EC):
                nc.tensor.matmul(
                    psk,
                    lhsT=w_all[:, ei, H:2 * H],
                    rhs=XT[:, blk, ei, :],
                    start=(ei == 0),
                    stop=(ei == EC - 1),
                )
            nc.vector.tensor_copy(kT[:, sl], psk)
            # odd chunks (local 1,3) -> partitions 64-127 of kodd at pair
            # columns [2*blk*128, (2*blk+2)*128)
            nc.scalar.dma_start(
                out=kodd[64:128, blk * 256:(blk + 1) * 256].rearrange(
                    "p (two c) -> p two c", c=128
                ),
                in_=kT[:, sl].rearrange("p (four c) -> p four c", c=128)[:, 1::2, :],
            )

        def emit_v(c):
            psv = psproj.tile([128, H], F32, tag="pv", name="psv")
            for ei in range(EC):
                nc.tensor.matmul(
                    psv,
                    lhsT=XT[:, c // 4, ei, (c % 4) * 128:(c % 4 + 1) * 128],
                    rhs=w_all[:, ei, 2 * H:3 * H],
                    start=(ei == 0),
                    stop=(ei == EC - 1),
                )
            nc.vector.tensor_copy(vA[:, c, 0:H], psv)

        # ---- score / softmax / ctx pipeline ----
        epool = ctx.enter_context(tc.tile_pool(name="epool", bufs=3))
        spool = ctx.enter_context(tc.tile_pool(name="spool", bufs=16))

        psctx_holder = {}
        widx_holder = {"i": 0}

        def emit_window(h, r, j, c):
            wt = wpool.tile([128, 1024], F32, tag="w", name="wt")
            for qb in range(2):
                col = h * 1024 + qb * 512
                if j == 0:
                    lhsT = kT[:, c * 128:(c + 1) * 128]
                    rhs = qT2[0:64, col:col + 512]
                else:
                    lhsT = kodd[64:128, r * 128:(r + 1) * 128]
                    rhs = qT2[64:128, col:col + 512]
                nc.tensor.matmul(
                    wt[:, qb * 512:(qb + 1) * 512],
                    lhsT=lhsT, rhs=rhs, start=True, stop=True,
                )
            et = epool.tile([128, 1024], BF16, tag="e", name="et")
            nc.scalar.activation(et, wt, AF.Exp, scale=SCALE)
            st = spool.tile([128, 1024], BF16, tag="s", name="st")
            # mask multiply: uint8 mask keeps DVE at 1x rate, so ~40% of
            # the windows run on the otherwise-idle GpSimd instead
            widx = widx_holder["i"]
            widx_holder["i"] += 1
            eng = nc.gpsimd if (widx * 13) // 32 != ((widx + 1) * 13) // 32 \
                else nc.vector
            eng.tensor_tensor(
                st, et, m_tile(c)[:, h * 1024:(h + 1) * 1024], ALU.mult
            )
            ctxall = psctx_holder["t"]
            for qb in range(2):
                col = h * 1024 + qb * 512
                nc.tensor.matmul(
                    ctxall[0:H + 1, col:col + 512],
                    lhsT=vA[:, c, 0:H + 1],
                    rhs=st[:, qb * 512:(qb + 1) * 512],
                    start=(r == 0 and j == 0),
                    stop=(r == SC // 2 - 1 and j == 1),
                )

        # Emission order = engine-queue order; each emit's deps follow the
        # DMA arrival order (ITa, XTb0, ITb, XTb1, XTb2, XTb3).
        emit_q(0, lambda: psproj.tile([64, 512], F32, tag="pk", name="psq"))
        emit_k(0)
        for c in range(0, 4):
            emit_v(c)
        emit_k(1)
        for c in range(4, 8):
            emit_v(c)
        emit_k(2)
        for c in range(8, 12):
            emit_v(c)
        emit_k(3)
        for c in range(12, 16):
            emit_v(c)
        proj_scope.close()

        psctx = ctx.enter_context(tc.tile_pool(name="psctx", bufs=1, space="PSUM"))
        psctx_holder["t"] = psctx.tile([128, S], F32, tag="ctxall", name="ctxall")

        for R in range(0, SC // 2, 2):
            for h in range(2):
                if R == 0 and h == 1:
                    # q half-b projection borrows score PSUM buffers; its
                    # MMs sit after the first four windows in the PE
                    # queue, by which time ITb has landed.
                    emit_q(1, lambda: wpool.tile([128, 1024], F32, tag="w",
                                                 name="psq_b")[0:64, 0:512])
                for r in (R, R + 1):
                    for j in range(2):
                        emit_window(h, r, j, 2 * r + j)

        for h in range(2):
            hs = slice(h * 1024, (h + 1) * 1024)
            nc.vector.tensor_copy(out_sb[:, hs], psctx_holder["t"][0:H + 1, hs])
            nc.sync.dma_start(out=dout.ap()[:, hs], in_=out_sb[:, hs])

    nc.compile()
    return nc


def get_program():
    if "nc" not in _cache:
        _cache["nc"] = _build_program()
    return _cache["nc"]


def make_in_maps(I, x, mask, Wq, bq, Wk, bk, Wv, bv):
    import ml_dtypes

    BF = ml_dtypes.bfloat16
    I = np.asarray(I, dtype=np.float32)
    x = np.asarray(x, dtype=np.float32)
    mask = np.asarray(mask, dtype=np.int32)
    Wpack = np.concatenate(
        [
            np.asarray(Wq, dtype=np.float32),
            np.asarray(Wk, dtype=np.float32),
            np.asarray(Wv, dtype=np.float32),
        ],
        axis=1,
    ).astype(BF)
    bq2 = np.tile(np.asarray(bq, np.float32).reshape(H, 1), (2, 1))

    def pack_blocks(a, nblk):
        # [S, E] -> [nblk, p, ec, S/nblk] partition-major seq-blocks of a^T
        w = S // nblk
        t = np.ascontiguousarray(a.T).astype(BF).reshape(EC, 128, nblk, w)
        return np.ascontiguousarray(t.transpose(2, 1, 0, 3))

    def pack_mask(m):
        # [S(q), S(k)] -> [p, chunk, q] of mask^T
        return np.ascontiguousarray(
            m.T.astype(np.uint8).reshape(SC, 128, S).transpose(1, 0, 2)
        )

    return [
        {
            "IT": pack_blocks(I[b], 2),
            "XT": pack_blocks(x[b], 4),
            "maskT": pack_mask(mask[b]),
            "Wpack": Wpack,
            "bq2": bq2,
        }
        for b in range(B)
    ]


def postprocess(raw, bv):
    """raw: [65, S] f32 (64 ctx rows + denominator). Returns [S, H] f32."""
    return (raw[0:H] / raw[H:H + 1]).T + np.asarray(bv, np.float32)


def kernel(I, x, mask, Wq, bq, Wk, bk, Wv, bv):
    nc = get_program()
    in_maps = make_in_maps(I, x, mask, Wq, bq, Wk, bk, Wv, bv)
    res = run_bass_kernel_spmd(nc, in_maps, list(range(N_CORES)))
    out = np.stack(
        [postprocess(res.results[b]["out"], bv) for b in range(B)], axis=0
    )
    return out.astype(np.float32)
